# revision 1
# baseline (speedup 1.0000x reference)
"""TRN2 Bass kernel for nn_Attention_90460601189287.

Causal multi-head attention (B=2, N=2048, D=1024, H=16) with spectral-norm
(power-iteration) scaled qkv/proj dense layers, on 8 NeuronCores.

Sharding: tensor-parallel over heads. Core c owns heads {2c, 2c+1}: it gets
the matching 128 columns of each of W_qkv's q/k/v blocks and the matching
128 rows of W_proj, computes attention for its heads over the full batch,
and produces a partial y = x_att @ W_proj_rows. The host sums the 8
partials (the gather step for row-sharded matmul).

The tiny spectral-norm power-iteration scales (two matvecs + norms over the
weights, ~0.01% of total FLOPs; identical math to the reference:
sigma = ||W^T normalize(W u)||) are computed on host in fp32 during input
sharding; the resulting scalar scales are applied on-device.

Per-core device program (SPMD — identical program, per-core weight slices),
software-pipelined per 512-token window so PE/ACT/DVE overlap across stages:
  A: PE-transpose x into x^T; qkv^T = W^T x accumulated over 8 dm-chunks
     (float32r matmuls: tf32 rate, ~2e-4 final accuracy).
  A2: PE-transpose v^T into V-natural [k,128] tiles whose cols 64:128 are
      ones so the PV matmul also emits the softmax denominator.
  B: S^T = K Q^T per 128-k-block; exp(S - 30) on ScalarE (constant shift
     replaces the row-max pass; exact after normalization since scores are
     O(1)); causal mask multiply on diagonal blocks; O^T accumulated in
     PSUM with denominator on partitions 64:127, partition-aligned with
     the numerator; normalize via reciprocal+mult.
  C: y_partial = x_att^T-blocks @ W_proj, stored per half-window.
"""
from contextlib import ExitStack

import numpy as np

import concourse.bass as bass
import concourse.mybir as mybir
from concourse.bass_utils import run_bass_kernel_spmd
from concourse.masks import make_identity
from concourse.tile import TileContext

F32 = mybir.dt.float32
F32R = mybir.dt.float32r

N_CORES = 8
BATCH = 2
NTOK = 4096      # flattened b*n
D = 1024
NH = 2           # heads per core
HD = 64
B = 2
NSEQ = 2048
WQ = 512         # token window
NW = NTOK // WQ
NWB = NSEQ // WQ
KB = 128
SHIFT = 30.0


def r(ap):
    return ap.bitcast(F32R)


# ---------------------------------------------------------------------------
# Workaround: this walrus build accepts at most ONE sync wait per
# instruction. Hoist extra waits onto single-wait NOPs inserted before.
# ---------------------------------------------------------------------------
def _split_sync_waits(nc, max_waits=1):
    for f in nc.m.functions:
        for blk in f.blocks:
            insts = blk.instructions
            out = []
            changed = False
            for inst in insts:
                si = inst.sync_info
                waits = list(si.on_wait) if si is not None else []
                if len(waits) > max_waits:
                    extra = waits[:-max_waits]
                    for i in range(0, len(extra), max_waits):
                        nop = mybir.InstNoOp(name=f"I-{nc.next_id()}", ins=[],
                                             outs=[], engine=inst.engine)
                        nop.sync_info = mybir.SyncInfo(
                            on_wait=extra[i:i + max_waits], on_update=[])
                        nc.register_instruction(nop, overwrite=True)
                        out.append(nop)
                    si.on_wait = waits[-max_waits:]
                    inst.sync_info = si
                    changed = True
                out.append(inst)
            if changed:
                blk.instructions = out


class _TileContextSplit(TileContext):
    def __exit__(self, exc_type, exc_value, traceback):
        ret = super().__exit__(exc_type, exc_value, traceback)
        if exc_type is None:
            _split_sync_waits(self.nc)
        return ret


def declare_params(nc):
    x = nc.declare_dram_parameter("x", [NTOK, D], F32R, isOutput=False)
    wq = nc.declare_dram_parameter("wq", [D, NH * HD], F32, isOutput=False)
    wk = nc.declare_dram_parameter("wk", [D, NH * HD], F32, isOutput=False)
    wv = nc.declare_dram_parameter("wv", [D, NH * HD], F32, isOutput=False)
    wp = nc.declare_dram_parameter("wp", [NH * HD, D], F32, isOutput=False)
    cqk = nc.declare_dram_parameter("cqk", [128, 1], F32, isOutput=False)
    cv = nc.declare_dram_parameter("cv", [128, 1], F32, isOutput=False)
    cp = nc.declare_dram_parameter("cp", [128, 1], F32, isOutput=False)
    mask = nc.declare_dram_parameter("mask", [128, 896], F32, isOutput=False)
    y = nc.declare_dram_parameter("y", [NTOK, D], F32, isOutput=True)
    return x, wq, wk, wv, wp, cqk, cv, cp, mask, y


def _build_body(nc, tc):
    mm = r
    tr = lambda ap: ap
    psum_bufs = dict(tp=2, qkv=1, s=2, o=1)
    xt_split = 5
    x, wq, wk, wv, wp, cqk, cv, cp, mask, y = declare_params(nc)

    ctx = ExitStack()
    with ctx:
        singles = ctx.enter_context(tc.tile_pool(name="singles", bufs=1))
        ident = singles.tile([128, 128], F32)
        make_identity(nc, ident)
        # f32r-rounded identity: lets the vT transposes (whose data inputs
        # are already f32r-rounded) run at 1.5 cyc/row instead of 2.0
        ident_r = singles.tile([128, 128], F32)
        nc.vector.tensor_copy(r(ident_r[:]), ident[:])

        xw_pool = ctx.enter_context(tc.tile_pool(name="xw", bufs=2))
        xw_pre = {}

        def load_xw(w):
            if w == 0:
                subs = []
                for t in range(4):
                    xw_s = xw_pool.tile([128, D], F32R, tag="xw0",
                                        name="xw_s", bufs=4)
                    nc.sync.dma_start(
                        out=xw_s[:],
                        in_=x[w * WQ + t * 128:w * WQ + (t + 1) * 128, :])
                    subs.append(xw_s)
                xw_pre[w] = subs
            else:
                xw_t = xw_pool.tile([128, 4, D], F32R, tag="xw", name="xw_t")
                nc.sync.dma_start(
                    out=xw_t[:],
                    in_=x[w * WQ:(w + 1) * WQ, :]
                        .rearrange("(t p) d -> p t d", p=128))
                xw_pre[w] = xw_t

        load_xw(0)
        mask_sb = singles.tile([128, 896], F32)
        nc.gpsimd.dma_start(out=mask_sb[:], in_=mask[:])
        cqk_sb = singles.tile([128, 1], F32)
        nc.gpsimd.dma_start(out=cqk_sb[:], in_=cqk[:])
        cv_sb = singles.tile([128, 1], F32)
        nc.gpsimd.dma_start(out=cv_sb[:], in_=cv[:])
        cp_sb = singles.tile([128, 1], F32)
        nc.gpsimd.dma_start(out=cp_sb[:], in_=cp[:])
        shift_sb = singles.tile([128, 1], F32)
        nc.gpsimd.memset(shift_sb[:], -SHIFT)
        ones_sb = singles.tile([128, 4 * HD], F32)
        nc.gpsimd.memset(ones_sb[:], 1.0)
        zeros_sb = singles.tile([128, 384], F32)
        nc.gpsimd.memset(zeros_sb[:], 0.0)

        # weights first (small; the first qkv matmuls need chunk 0), then
        # the first x windows. Rounding copies are chunked so qkv chunk dm
        # is ready as soon as its three 64KB slices have landed.
        wq_sb = singles.tile([128, D], F32)
        wk_sb = singles.tile([128, D], F32)
        wv_sb = singles.tile([128, D], F32)
        wp_sb = singles.tile([128, D], F32)
        with tc.tile_pool(name="wst", bufs=1) as wst:
            wq_st = wst.tile([128, D], F32)
            wk_st = wst.tile([128, D], F32)
            wv_st = wst.tile([128, D], F32)
            wp_st = wst.tile([128, D], F32)
            for w_dram, w_st, w_fin in ((wq, wq_st, wq_sb),
                                         (wk, wk_st, wk_sb),
                                         (wv, wv_st, wv_sb)):
                nc.sync.dma_start(
                    out=w_st.rearrange("p (c m) -> p c m", c=8),
                    in_=w_dram.rearrange("(c p) m -> p c m", p=128))
                nc.vector.tensor_copy(mm(w_fin[:]), w_st[:])
            nc.sync.dma_start(out=wp_st[:], in_=wp[:])
            nc.vector.tensor_scalar_mul(mm(wp_sb[:]), wp_st[:], cp_sb[:, 0:1])

            load_xw(1)

        # per-window qkv^T and attention-output^T tiles (window granularity
        # is what lets stage B start while stage A is still running)
        qTw = [singles.tile([128, WQ], F32, name=f"qT_{w}") for w in range(NW)]
        kTw = [singles.tile([128, WQ], F32, name=f"kT_{w}") for w in range(NW)]
        vTw = [singles.tile([128, WQ], F32, name=f"vT_{w}") for w in range(NW)]
        xaw = [singles.tile([128, WQ], F32, name=f"xa_{w}") for w in range(NW)]
        # V natural layout per (head, batch, group of 4 k-blocks):
        # [128, 4, 128]; cols 64:128 all-ones (denominator trick)
        vnat = [[[singles.tile([128, 4, 2 * HD], F32, name=f"vn_{h}_{b}_{g}")
                  for g in range(NWB)] for b in range(B)] for h in range(NH)]

        # one PSUM pool for the whole kernel: tp1+qkv3+s2+o2 = 8 banks
        ps = ctx.enter_context(tc.tile_pool(name="ps", bufs=1, space="PSUM"))
        xt_pool = ctx.enter_context(tc.tile_pool(name="xt", bufs=3))
        a_pool = ctx.enter_context(tc.tile_pool(name="apool", bufs=6))
        den_pool = ctx.enter_context(tc.tile_pool(name="denpool", bufs=3))
        y_pool = ctx.enter_context(tc.tile_pool(name="ypool", bufs=2))

        # ---- Stage A for one token window, as a list of chunk closures so
        # the emitter can interleave them into stage B's PE bubbles ----
        def stage_a_ops(w):
            state = {}

            def start():
                if w not in xw_pre:
                    load_xw(w)
                state["xw"] = xw_pre.pop(w)
                state["qkv"] = [
                    ps.tile([128, WQ], F32, tag=f"qkv{i}", name=f"qkv_ps{i}",
                            bufs=psum_bufs["qkv"]) for i in range(3)]

            def mk_dm(dm):
                def op():
                    xw_t = state["xw"]
                    sub = isinstance(xw_t, list)
                    tp_ps = ps.tile([128, WQ], F32, tag="tp", name="tp_ps",
                                    bufs=psum_bufs["tp"])
                    for t in range(4):
                        xsrc = (xw_t[t][:, dm * 128:(dm + 1) * 128] if sub
                                else xw_t[:, t, dm * 128:(dm + 1) * 128])
                        nc.tensor.transpose(
                            r(tp_ps[:, t * 128:(t + 1) * 128]),
                            xsrc, r(ident_r[:]))
                    xt_t = xt_pool.tile([128, WQ], F32, tag="xt", name="xt_t")
                    if xt_split and dm % xt_split == 0:
                        nc.scalar.copy(mm(xt_t[:]), tp_ps[:])
                    else:
                        nc.vector.tensor_copy(mm(xt_t[:]), tp_ps[:])
                    for i, w_sb in enumerate((wq_sb, wk_sb, wv_sb)):
                        nc.tensor.matmul(state["qkv"][i][:],
                                         mm(w_sb[:, dm * 128:(dm + 1) * 128]),
                                         mm(xt_t[:]),
                                         start=(dm == 0), stop=(dm == 7))
                return op

            def copies():
                qkv_ps = state["qkv"]
                nc.vector.tensor_scalar_mul(mm(qTw[w][:]), qkv_ps[0][:],
                                            cqk_sb[:, 0:1])
                nc.scalar.copy(mm(kTw[w][:]), qkv_ps[1][:])
                nc.vector.tensor_scalar_mul(mm(vTw[w][:]), qkv_ps[2][:],
                                            cv_sb[:, 0:1])

            def mk_a2(h):
                def op():
                    b, g = divmod(w, NWB)
                    vn = vnat[h][b][g]
                    nc.vector.tensor_copy(
                        mm(vn[:, :, HD:2 * HD]),
                        ones_sb.rearrange("p (g d) -> p g d",
                                          g=4)[:, 0:4, 0:HD])
                    vp = ps.tile([128, 4, HD], F32, tag="tp", name="vp",
                                 bufs=psum_bufs["tp"])
                    for j in range(4):
                        nc.tensor.transpose(
                            r(vp[:, j, :]),
                            r(vTw[w][h * HD:(h + 1) * HD,
                                     j * KB:(j + 1) * KB]),
                            r(ident_r[h * HD:(h + 1) * HD,
                                      h * HD:(h + 1) * HD]))
                    nc.vector.tensor_copy(mm(vn[:, :, 0:HD]), vp[:])
                return op

            return ([start] + [mk_dm(dm) for dm in range(8)] + [copies] +
                    [mk_a2(h) for h in range(NH)])

        def stage_a(w):
            for op in stage_a_ops(w):
                op()

        # ---- Stage B for one (batch, q-window): both heads, interleaved by
        # k-block so two independent S->exp->PV chains hide the exp latency ----
        def stage_b(b, g, c_ops=()):
            c_iter = iter(c_ops)
            n_units = NH * (g + 1) * (WQ // KB)
            n_c = len(c_ops)
            emitted_c = 0
            done_units = 0
            for h in range(NH):
                hs = slice(h * HD, (h + 1) * HD)
                o_ps = ps.tile([2 * HD, WQ], F32, tag="o", name="o_ps",
                               bufs=psum_bufs["o"])
                nkb = (g + 1) * (WQ // KB)
                for kb in range(nkb):
                    kw = b * NWB + kb // 4       # global window of k block
                    ko = (kb % 4) * KB
                    sq = max(0, kb * KB - g * WQ)  # first causally-valid col
                    s_ps = ps.tile([128, WQ], F32, tag="s", name="s_ps",
                                   bufs=psum_bufs["s"])
                    nc.tensor.matmul(s_ps[:, sq:WQ],
                                     mm(kTw[kw][hs, ko:ko + KB]),
                                     mm(qTw[b * NWB + g][hs, sq:WQ]),
                                     start=True, stop=True)
                    a_t = a_pool.tile([128, WQ], F32, tag="a", name="a_t")
                    s = kb * KB - g * WQ
                    if s >= 0:  # diagonal block: causal masking. Columns
                        # [0:s] are entirely above the diagonal: zero them
                        # and restrict exp+mask to the valid range [s:512].
                        if s > 0:
                            nc.vector.tensor_copy(mm(a_t[:, 0:s]),
                                                  zeros_sb[:, 0:s])
                        nc.scalar.activation(mm(a_t[:, s:WQ]),
                                             s_ps[:, s:WQ],
                                             mybir.ActivationFunctionType.Exp,
                                             bias=shift_sb[:, 0:1], scale=1.0)
                        nc.vector.tensor_tensor(
                            out=mm(a_t[:, s:WQ]), in0=a_t[:, s:WQ],
                            in1=mask_sb[:, 384:896 - s],
                            op=mybir.AluOpType.mult)
                    else:
                        nc.scalar.activation(mm(a_t[:]), s_ps[:],
                                             mybir.ActivationFunctionType.Exp,
                                             bias=shift_sb[:, 0:1], scale=1.0)
                    nc.tensor.matmul(o_ps[:, sq:WQ] if sq else o_ps[:],
                                     mm(vnat[h][b][kb // 4][:, kb % 4, :]),
                                     mm(a_t[:, sq:WQ] if sq else a_t[:]),
                                     start=(kb == 0), stop=(kb == nkb - 1))
                    done_units += 1
                    # sprinkle the previous window's proj work into the
                    # S->exp->PV bubbles on PE
                    if n_c:
                        want = done_units * n_c // n_units
                        while emitted_c < want:
                            next(c_iter)()
                            emitted_c += 1
                den_sb = den_pool.tile([HD, WQ], F32, tag="den",
                                       name="den_sb")
                nc.vector.reciprocal(den_sb[:], o_ps[HD:2 * HD, :])
                nc.vector.tensor_tensor(
                    out=mm(xaw[b * NWB + g][hs, :]), in0=o_ps[0:HD, :],
                    in1=den_sb[:], op=mybir.AluOpType.mult)

        # ---- Stage C for one token window: proj partial for 4 n-blocks ----
        def stage_c_ops(w):
            ops = []
            state = {}

            def mk_mm(half, j, nb, cc):
                def op():
                    if j == 0 and cc == 0:
                        state[half] = y_pool.tile([128, 2, D], F32, tag="y",
                                                  name="y_sb")
                    y_sb = state[half]
                    yp = ps.tile([128, 512], F32, tag="s", name=f"yp{cc}",
                                 bufs=psum_bufs["s"])
                    nc.tensor.matmul(
                        yp[:],
                        mm(xaw[nb // 4][:, (nb % 4) * 128:
                                        (nb % 4 + 1) * 128]),
                        mm(wp_sb[:, cc * 512:(cc + 1) * 512]),
                        start=True, stop=True)
                    if cc == 0:
                        nc.scalar.copy(y_sb[:, j, 0:512], yp[:])
                    else:
                        nc.vector.tensor_copy(y_sb[:, j, 512:1024], yp[:])
                    if j == 1 and cc == 1:
                        nb0 = 4 * w + 2 * half
                        nc.sync.dma_start(
                            out=y[nb0 * 128:(nb0 + 2) * 128, :]
                                .rearrange("(n p) d -> p n d", p=128),
                            in_=y_sb[:])
                return op

            for half in range(2):
                nb0 = 4 * w + 2 * half
                for j, nb in enumerate((nb0, nb0 + 1)):
                    for cc in range(2):
                        ops.append(mk_mm(half, j, nb, cc))
            return ops

        # ---- software-pipelined emission: A(w), then B(w) with the
        # previous window's proj matmuls interleaved into its bubbles ----
        for w in range(NW):
            stage_a(w)
            b, g = divmod(w, NWB)
            stage_b(b, g)
            for op in stage_c_ops(w):
                op()




def _make_mask():
    p = np.arange(128)[:, None]
    j = np.arange(896)[None, :]
    return (j >= p + 384).astype(np.float32)


def _host_scales(W_qkv, u_qkv, sigma_qkv, W_proj, u_proj, sigma_proj):
    """Power-iteration spectral norm in fp32, exactly as the reference:
    v = normalize(W u); sigma = ||W^T v||."""
    def sig(W, u):
        v = (W @ u).astype(np.float32)
        v = v / np.float32(np.linalg.norm(v))
        u2 = (W.T @ v).astype(np.float32)
        return np.float32(np.linalg.norm(u2))
    c_qkv = np.float32(sigma_qkv[0]) / sig(W_qkv, u_qkv)
    c_proj = np.float32(sigma_proj[0]) / sig(W_proj, u_proj)
    return np.float32(c_qkv), np.float32(c_proj)


def make_in_maps(batch, W_qkv, u_qkv, sigma_qkv, W_proj, u_proj, sigma_proj):
    batch = np.asarray(batch, np.float32)
    W_qkv = np.asarray(W_qkv, np.float32)
    u_qkv = np.asarray(u_qkv, np.float32)
    sigma_qkv = np.asarray(sigma_qkv, np.float32)
    W_proj = np.asarray(W_proj, np.float32)
    u_proj = np.asarray(u_proj, np.float32)
    sigma_proj = np.asarray(sigma_proj, np.float32)
    x = np.ascontiguousarray(batch.reshape(NTOK, D))
    # pre-round x to the f32r (tf32-like) grid: the device rounds it at the
    # x^T staging copy anyway, so accuracy is unchanged (~2e-4) and the
    # f32r-typed DMA satisfies the verifier, letting the 256 x-transposes
    # run at 1.5 instead of 2.0 cycles/row
    u = x.view(np.uint32)
    u += ((u >> 13) & 1) + np.uint32((1 << 12) - 1)
    u &= np.uint32(~((1 << 13) - 1) & 0xFFFFFFFF)
    c_qkv, c_proj = _host_scales(W_qkv, u_qkv, sigma_qkv,
                                 W_proj, u_proj, sigma_proj)
    scale = np.float32(HD ** -0.5)
    mask = _make_mask()
    in_maps = []
    for c in range(N_CORES):
        cs = slice(128 * c, 128 * (c + 1))
        in_maps.append({
            "x": x,
            "wq": np.ascontiguousarray(W_qkv[:, cs]),
            "wk": np.ascontiguousarray(W_qkv[:, 1024 + 128 * c:
                                              1024 + 128 * (c + 1)]),
            "wv": np.ascontiguousarray(W_qkv[:, 2048 + 128 * c:
                                              2048 + 128 * (c + 1)]),
            "wp": np.ascontiguousarray(W_proj[cs, :]),
            "cqk": np.full((128, 1), c_qkv * c_qkv * scale, np.float32),
            "cv": np.full((128, 1), c_qkv, np.float32),
            "cp": np.full((128, 1), c_proj, np.float32),
            "mask": mask,
        })
    return in_maps


_NC_CACHE = None


def build_nc():
    global _NC_CACHE
    if _NC_CACHE is None:
        nc = bass.Bass("TRN2", target_bir_lowering=False, debug=False,
                       num_devices=N_CORES)
        with _TileContextSplit(nc) as tc:
            _build_body(nc, tc)
        _NC_CACHE = nc
    return _NC_CACHE


def kernel(batch, W_qkv, u_qkv, sigma_qkv, W_proj, u_proj, sigma_proj):
    in_maps = make_in_maps(batch, W_qkv, u_qkv, sigma_qkv,
                           W_proj, u_proj, sigma_proj)
    nc = build_nc()
    res = run_bass_kernel_spmd(nc, in_maps, list(range(N_CORES)))
    y = np.zeros((NTOK, D), np.float64)
    for c in range(N_CORES):
        y += res.results[c]["y"].astype(np.float64)
    return y.astype(np.float32).reshape(BATCH, NSEQ, D)



# revision 8
# speedup vs baseline: 1.4611x; 1.4611x over previous
"""TRN2 Bass kernel for nn_Attention_90460601189287.

Causal multi-head attention (B=2, N=2048, D=1024, H=16) with spectral-norm
(power-iteration) scaled qkv/proj dense layers, on 8 NeuronCores.

Sharding: tensor-parallel over heads. Core c owns heads {2c, 2c+1}: it gets
the matching 128 columns of each of W_qkv's q/k/v blocks and the matching
128 rows of W_proj, computes attention for its heads over the full batch,
and produces a partial y = x_att @ W_proj_rows (fp16). The host sums the 8
partials in fp32 (the gather step for row-sharded matmul).

The tiny spectral-norm power-iteration scales (identical math to the
reference: sigma = ||W^T normalize(W u)||) are computed on host in fp32 and
folded into the weights / a single per-side scale gamma.

Device program (SPMD; per-core weight slices), per 512-token window w:
  A: qkv^T = W^T x^T, with x^T provided by the host (fp8 for q/k, fp16 for
     v) so no on-device transposes of x are needed. q,k run as fp8
     DoubleRow matmuls (2 d-chunks per pass, 0.5 cyc/row); v in fp16.
     q^T/k^T are stored as fp8 with a zero second slot so the attention
     S matmul can also use DoubleRow; v^T is PE-transposed into V-natural
     tiles whose cols 64:127 are ones (the PV matmul then also emits the
     softmax denominator for free).
  B: per (head, 2-k-block group): S^T = K Q^T via one fp8 DoubleRow matmul
     per k-block into a 2-bank PSUM group; one Exp activation per group
     (no max pass / shift: scores are O(1) so exp() is in fp16 range);
     causal triangle mask multiply on diagonal blocks; O^T accumulated in
     PSUM with the denominator on partitions 64:127; normalize via
     reciprocal+mult.
  C: y_partial = x_att^T-blocks @ W_proj, staged via Pool copies, fp16 DMA.

Engines are in-order, so stage A(w+1) and C(w-1) ops are interleaved into
stage B(w)'s exp-latency bubbles explicitly (ACT is the critical engine:
it runs only the 80 grouped exps).
"""
from contextlib import ExitStack

import numpy as np

import concourse.bass as bass
import concourse.mybir as mybir
from concourse.bass_utils import run_bass_kernel_spmd
from concourse.masks import make_identity
from concourse.tile import TileContext

F32 = mybir.dt.float32
F16 = mybir.dt.float16
F8 = mybir.dt.float8e4

N_CORES = 8
BATCH = 2
NTOK = 4096      # flattened b*n
D = 1024
NH = 2           # heads per core
HD = 64
B = 2
NSEQ = 2048
WQ = 512         # token window
NW = NTOK // WQ
NWB = NSEQ // WQ
KB = 128
BETA = 8.0       # host-side fp8 weight pre-scale (keeps W in fp8e4m3 range)

DR = mybir.MatmulPerfMode.DoubleRow
EXP = mybir.ActivationFunctionType.Exp
MUL = mybir.AluOpType.mult


# ---------------------------------------------------------------------------
# Workaround: this walrus build accepts at most ONE sync wait per
# instruction. Hoist extra waits onto single-wait NOPs inserted before.
# ---------------------------------------------------------------------------
def _split_sync_waits(nc, max_waits=1):
    for f in nc.m.functions:
        for blk in f.blocks:
            insts = blk.instructions
            out = []
            changed = False
            for inst in insts:
                si = inst.sync_info
                waits = list(si.on_wait) if si is not None else []
                if len(waits) > max_waits:
                    extra = waits[:-max_waits]
                    for i in range(0, len(extra), max_waits):
                        nop = mybir.InstNoOp(name=f"I-{nc.next_id()}", ins=[],
                                             outs=[], engine=inst.engine)
                        nop.sync_info = mybir.SyncInfo(
                            on_wait=extra[i:i + max_waits], on_update=[])
                        nc.register_instruction(nop, overwrite=True)
                        out.append(nop)
                    si.on_wait = waits[-max_waits:]
                    inst.sync_info = si
                    changed = True
                out.append(inst)
            if changed:
                blk.instructions = out


class _TileContextSplit(TileContext):
    def __exit__(self, exc_type, exc_value, traceback):
        ret = super().__exit__(exc_type, exc_value, traceback)
        if exc_type is None:
            _split_sync_waits(self.nc)
        return ret


def declare_params(nc):
    xb = nc.declare_dram_parameter("xb", [D, NTOK], F16, isOutput=False)
    x8 = nc.declare_dram_parameter("x8", [D, NTOK], F8, isOutput=False)
    w8q = nc.declare_dram_parameter("w8q", [D, NH * HD], F8, isOutput=False)
    w8k = nc.declare_dram_parameter("w8k", [D, NH * HD], F8, isOutput=False)
    wvh = nc.declare_dram_parameter("wvh", [D, NH * HD], F16, isOutput=False)
    wph = nc.declare_dram_parameter("wph", [NH * HD, D], F16, isOutput=False)
    gam = nc.declare_dram_parameter("gam", [128, 1], F32, isOutput=False)
    tri = nc.declare_dram_parameter("tri", [128, KB], F16, isOutput=False)
    y = nc.declare_dram_parameter("y", [NTOK, D], F16, isOutput=True)
    return xb, x8, w8q, w8k, wvh, wph, gam, tri, y


def _build_body(nc, tc):
    xb, x8, w8q, w8k, wvh, wph, gam, tri, y = declare_params(nc)

    ctx = ExitStack()
    with ctx:
        singles = ctx.enter_context(tc.tile_pool(name="singles", bufs=1))

        # --- constants / weights to SBUF ---
        gam_sb = singles.tile([128, 1], F32)
        nc.gpsimd.dma_start(out=gam_sb[:], in_=gam[:])
        tri_sb = singles.tile([128, KB], F16)
        nc.gpsimd.dma_start(out=tri_sb[:], in_=tri[:])

        ident = singles.tile([128, 128], F32)
        make_identity(nc, ident)
        ident_h = singles.tile([128, 128], F16)
        nc.vector.tensor_copy(ident_h[:], ident[:])

        w8q_sb = singles.tile([128, 8, NH * HD], F8)
        nc.sync.dma_start(out=w8q_sb[:],
                          in_=w8q.rearrange("(c p) m -> p c m", p=128))
        w8k_sb = singles.tile([128, 8, NH * HD], F8)
        nc.sync.dma_start(out=w8k_sb[:],
                          in_=w8k.rearrange("(c p) m -> p c m", p=128))
        wvh_sb = singles.tile([128, 8, NH * HD], F16)
        nc.sync.dma_start(out=wvh_sb[:],
                          in_=wvh.rearrange("(c p) m -> p c m", p=128))
        wph_sb = singles.tile([128, D], F16)
        nc.sync.dma_start(out=wph_sb[:], in_=wph[:])

        # --- persistent per-window tiles ---
        # q^T/k^T as fp8 with a zero slot (dim1) so the S matmul can run in
        # DoubleRow mode (2 contraction tiles per pass; slot 1 contributes 0)
        qT8 = [singles.tile([128, 2, WQ], F8, name=f"qT8_{w}")
               for w in range(NW)]
        kT8 = [singles.tile([128, 2, WQ], F8, name=f"kT8_{w}")
               for w in range(NW)]
        vTw = [singles.tile([128, WQ], F16, name=f"vT_{w}") for w in range(NW)]
        xaw = [singles.tile([128, WQ], F16, name=f"xa_{w}") for w in range(NW)]
        # V natural layout per (head, batch, k-window): [128 k, 4 kb, v|ones]
        vnat = [[[singles.tile([128, 4, 2 * HD], F16, name=f"vn_{h}_{b}_{g}")
                  for g in range(NWB)] for b in range(B)] for h in range(NH)]

        # zero the fp8 DoubleRow padding slots and the all-ones denominator
        # columns once, on Pool (window-major so window 0 unblocks first)
        for w in range(NW):
            b, g = divmod(w, NWB)
            nc.gpsimd.memset(qT8[w][:, 1, :], 0.0)
            nc.gpsimd.memset(kT8[w][:, 1, :], 0.0)
            for h in range(NH):
                nc.gpsimd.memset(vnat[h][b][g][:, :, HD:2 * HD], 1.0)

        # --- pools ---
        # PSUM budget (8 banks): qkv 1 + vp 1 + s 2x2 + o 2 = 8;
        # stage C's yp groups share the "s" tag ring.
        ps = ctx.enter_context(tc.tile_pool(name="ps", bufs=1, space="PSUM"))
        x8_pool = ctx.enter_context(tc.tile_pool(name="x8p", bufs=2))
        xb_pool = ctx.enter_context(tc.tile_pool(name="xbp", bufs=2))
        a_pool = ctx.enter_context(tc.tile_pool(name="apool", bufs=6))
        den_pool = ctx.enter_context(tc.tile_pool(name="denpool", bufs=3))
        y_pool = ctx.enter_context(tc.tile_pool(name="ypool", bufs=3))

        x8_pre = {}
        xb_pre = {}

        def load_x(w):
            ws = slice(w * WQ, (w + 1) * WQ)
            x8_t = x8_pool.tile([128, 8, WQ], F8, tag="x8", name="x8_t")
            nc.sync.dma_start(
                out=x8_t[:],
                in_=x8[:, ws].rearrange("(c p) n -> p c n", p=128))
            x8_pre[w] = x8_t
            xb_t = xb_pool.tile([128, 8, WQ], F16, tag="xb", name="xb_t")
            nc.sync.dma_start(
                out=xb_t[:],
                in_=xb[:, ws].rearrange("(c p) n -> p c n", p=128))
            xb_pre[w] = xb_t

        load_x(0)
        load_x(1)

        # ---- Stage A for one token window, as a list of closures so the
        # emitter can interleave them into stage B's exp-latency bubbles ----
        def ops_A(w):
            st = {}
            ops = []

            def op_load():
                if w + 1 < NW and w + 1 not in x8_pre:
                    load_x(w + 1)
            ops.append(op_load)

            def mk_qk(w8_sb, dst):
                def op_mm():
                    p = ps.tile([128, WQ], F32, tag="qkv", name="qkp")
                    for cp in range(4):
                        nc.tensor.matmul(
                            p[:], w8_sb[:, 2 * cp:2 * cp + 2, :],
                            x8_pre[w][:, 2 * cp:2 * cp + 2, :],
                            start=(cp == 0), stop=(cp == 3), perf_mode=DR)
                    st['p'] = p

                def op_cp():
                    nc.vector.tensor_scalar_mul(dst[:, 0, :], st['p'][:],
                                                gam_sb[:, 0:1])
                return [op_mm, op_cp]

            ops += mk_qk(w8q_sb, qT8[w])
            ops += mk_qk(w8k_sb, kT8[w])

            def op_v(half):
                def op():
                    if half == 0:
                        st['vp'] = ps.tile([128, WQ], F32, tag="qkv",
                                           name="vps")
                    p = st['vp']
                    for c in range(4 * half, 4 * half + 4):
                        nc.tensor.matmul(
                            p[:], wvh_sb[:, c, :], xb_pre[w][:, c, :],
                            start=(c == 0), stop=(c == 7))
                return op
            ops.append(op_v(0))
            ops.append(op_v(1))

            def op_vc():
                nc.vector.tensor_copy(vTw[w][:], st['vp'][:])
                x8_pre.pop(w)
                xb_pre.pop(w)
            ops.append(op_vc)

            def mk_a2(h):
                def op():
                    b, g = divmod(w, NWB)
                    vn = vnat[h][b][g]
                    vp2 = ps.tile([128, 4, HD], F16, tag="vp", name="vp2")
                    for j in range(4):
                        nc.tensor.transpose(
                            vp2[:, j, :],
                            vTw[w][h * HD:(h + 1) * HD,
                                   j * KB:(j + 1) * KB],
                            ident_h[h * HD:(h + 1) * HD,
                                    h * HD:(h + 1) * HD])
                    nc.vector.tensor_copy(vn[:, :, 0:HD], vp2[:])
                return op
            ops.append(mk_a2(0))
            ops.append(mk_a2(1))
            return ops

        # ---- Stage C for one token window: proj partials, 4 n-blocks ----
        def ops_C(w):
            st = {}
            ops = []
            for nb_loc in range(4):
                def op_mm(nb_loc=nb_loc):
                    yp = ps.tile([128, 2, WQ], F32, tag="s", name="yp",
                                 bufs=2)
                    for cc in range(2):
                        nc.tensor.matmul(
                            yp[:, cc, :],
                            xaw[w][:, nb_loc * 128:(nb_loc + 1) * 128],
                            wph_sb[:, cc * WQ:(cc + 1) * WQ],
                            start=True, stop=True)
                    st[nb_loc] = yp

                def op_st(nb_loc=nb_loc):
                    y_sb = y_pool.tile([128, D], F16, tag="y", name="y_sb")
                    nc.vector.tensor_copy(y_sb[:], st[nb_loc][:])
                    nb = 4 * w + nb_loc
                    nc.sync.dma_start(out=y[nb * 128:(nb + 1) * 128, :],
                                      in_=y_sb[:])
                ops.append(op_mm)
                ops.append(op_st)
            return ops

        # ---- Stage B for one (batch, q-window) with filler interleave ----
        def stage_B(b, g, filler):
            w = b * NWB + g
            n_pairs = 2 * (g + 1)
            n_f = len(filler)
            emitted = 0
            o_ps = [ps.tile([128, WQ], F32, tag="o", name=f"o_ps{h}",
                            bufs=2) for h in range(NH)]

            def emit_pv(u, a_ts):
                kw, j = divmod(u, 2)
                diag = (kw == g)
                for h in range(NH):
                    a_t = a_ts[h]
                    if diag:
                        for t in range(2):
                            c0 = (2 * j + t) * KB
                            nc.gpsimd.tensor_tensor(
                                out=a_t[:, t, c0:c0 + KB],
                                in0=a_t[:, t, c0:c0 + KB],
                                in1=tri_sb[:], op=MUL)
                    for t in range(2):
                        kloc = 2 * j + t
                        sq = kloc * KB if diag else 0
                        nc.tensor.matmul(
                            o_ps[h][:, sq:WQ] if sq else o_ps[h][:],
                            vnat[h][b][kw][:, kloc, :],
                            a_t[:, t, sq:WQ] if sq else a_t[:, t, :],
                            start=(u == 0 and t == 0),
                            stop=(u == n_pairs - 1 and t == 1))

            # software pipeline: emit pair u's S+exp, then pair u-1's PVs, so
            # the in-order PE queue never waits on an exp that was just issued
            pend = None
            for u in range(n_pairs):
                kw, j = divmod(u, 2)
                kwin = b * NWB + kw
                a_ts = [None, None]
                for h in range(NH):
                    hs = slice(h * HD, (h + 1) * HD)
                    s_grp = ps.tile([128, 2, WQ], F32, tag="s", name="s_grp",
                                    bufs=2)
                    for t in range(2):
                        kloc = 2 * j + t
                        nc.tensor.matmul(
                            s_grp[:, t, :],
                            kT8[kwin][hs, :, kloc * KB:(kloc + 1) * KB],
                            qT8[w][hs, :, :],
                            start=True, stop=True, perf_mode=DR)
                    a_t = a_pool.tile([128, 2, WQ], F16, tag="a", name="a_t")
                    nc.scalar.activation(a_t[:], s_grp[:], EXP)
                    a_ts[h] = a_t
                # filler into the exp latency bubble
                want = (u + 1) * n_f // n_pairs
                while emitted < want:
                    filler[emitted]()
                    emitted += 1
                if pend is not None:
                    emit_pv(*pend)
                pend = (u, a_ts)
            while emitted < n_f:
                filler[emitted]()
                emitted += 1
            emit_pv(*pend)
            for h in range(NH):
                hs = slice(h * HD, (h + 1) * HD)
                den = den_pool.tile([HD, WQ], F32, tag="den", name="den")
                nc.vector.reciprocal(den[:], o_ps[h][HD:2 * HD, :])
                nc.vector.tensor_tensor(out=xaw[w][hs, :],
                                        in0=o_ps[h][0:HD, :],
                                        in1=den[:], op=MUL)

        def riffle(a, c):
            out = []
            n = max(len(a), len(c))
            for i in range(n):
                if i < len(a):
                    out.append(a[i])
                if i < len(c):
                    out.append(c[i])
            return out

        # ---- software-pipelined emission ----
        for op in ops_A(0):
            op()
        for w in range(NW):
            b, g = divmod(w, NWB)
            filler = riffle(ops_A(w + 1) if w + 1 < NW else [],
                            ops_C(w - 1) if w >= 1 else [])
            stage_B(b, g, filler)
        for op in ops_C(NW - 1):
            op()


def _host_scales(W_qkv, u_qkv, sigma_qkv, W_proj, u_proj, sigma_proj):
    """Power-iteration spectral norm in fp32, exactly as the reference:
    v = normalize(W u); sigma = ||W^T v||."""
    def sig(W, u):
        v = (W @ u).astype(np.float32)
        v = v / np.float32(np.linalg.norm(v))
        u2 = (W.T @ v).astype(np.float32)
        return np.float32(np.linalg.norm(u2))
    c_qkv = np.float32(sigma_qkv[0]) / sig(W_qkv, u_qkv)
    c_proj = np.float32(sigma_proj[0]) / sig(W_proj, u_proj)
    return np.float32(c_qkv), np.float32(c_proj)


def _make_tri():
    p = np.arange(128)[:, None]
    q = np.arange(KB)[None, :]
    return (q >= p).astype(np.float32)


def make_in_maps(batch, W_qkv, u_qkv, sigma_qkv, W_proj, u_proj, sigma_proj):
    import ml_dtypes
    f16 = np.float16
    f8 = ml_dtypes.float8_e4m3
    batch = np.asarray(batch, np.float32)
    W_qkv = np.asarray(W_qkv, np.float32)
    u_qkv = np.asarray(u_qkv, np.float32)
    sigma_qkv = np.asarray(sigma_qkv, np.float32)
    W_proj = np.asarray(W_proj, np.float32)
    u_proj = np.asarray(u_proj, np.float32)
    sigma_proj = np.asarray(sigma_proj, np.float32)
    c_qkv, c_proj = _host_scales(W_qkv, u_qkv, sigma_qkv,
                                 W_proj, u_proj, sigma_proj)
    xT = np.ascontiguousarray(batch.reshape(NTOK, D).T)
    xb_h = xT.astype(f16)
    x8_h = xT.astype(f8)
    # S needs scale c^2/sqrt(hd); gamma is the per-side share applied at the
    # q^T/k^T PSUM->SBUF copies (fp8 weights carry a BETA pre-scale to stay
    # in e4m3's normal range)
    # (gamma*BETA)^2 == c^2 / sqrt(hd)  =>  gamma = c / (BETA * hd^(1/4))
    gamma = np.float32(c_qkv / (BETA * HD ** 0.25))
    tri = np.ascontiguousarray(_make_tri().astype(f16))
    in_maps = []
    for c in range(N_CORES):
        cs = slice(128 * c, 128 * (c + 1))
        w8q_h = np.ascontiguousarray((W_qkv[:, cs] * BETA).astype(f8))
        w8k_h = np.ascontiguousarray(
            (W_qkv[:, 1024 + 128 * c:1024 + 128 * (c + 1)] * BETA).astype(f8))
        wvh_h = np.ascontiguousarray(
            (W_qkv[:, 2048 + 128 * c:2048 + 128 * (c + 1)] * c_qkv)
            .astype(f16))
        wph_h = np.ascontiguousarray((W_proj[cs, :] * c_proj).astype(f16))
        in_maps.append({
            "xb": xb_h,
            "x8": x8_h,
            "w8q": w8q_h,
            "w8k": w8k_h,
            "wvh": wvh_h,
            "wph": wph_h,
            "gam": np.full((128, 1), gamma, np.float32),
            "tri": tri,
        })
    return in_maps


_NC_CACHE = None


def build_nc():
    global _NC_CACHE
    if _NC_CACHE is None:
        nc = bass.Bass("TRN2", target_bir_lowering=False, debug=False,
                       num_devices=N_CORES)
        with _TileContextSplit(nc) as tc:
            _build_body(nc, tc)
        _NC_CACHE = nc
    return _NC_CACHE


def kernel(batch, W_qkv, u_qkv, sigma_qkv, W_proj, u_proj, sigma_proj):
    in_maps = make_in_maps(batch, W_qkv, u_qkv, sigma_qkv,
                           W_proj, u_proj, sigma_proj)
    nc = build_nc()
    res = run_bass_kernel_spmd(nc, in_maps, list(range(N_CORES)))
    y = np.zeros((NTOK, D), np.float32)
    for c in range(N_CORES):
        y += res.results[c]["y"].astype(np.float32)
    return y.reshape(BATCH, NSEQ, D)


# revision 65
# speedup vs baseline: 1.5468x; 1.0587x over previous
"""TRN2 Bass kernel for nn_Attention_90460601189287.

Causal multi-head attention (B=2, N=2048, D=1024, H=16) with spectral-norm
(power-iteration) scaled qkv/proj dense layers, on 8 NeuronCores.

Sharding: tensor-parallel over heads. Core c owns heads {2c, 2c+1}: it gets
the matching 128 columns of each of W_qkv's q/k/v blocks and the matching
128 rows of W_proj, computes attention for its heads over the full batch,
and produces a partial y = x_att @ W_proj_rows (fp16). The host sums the 8
partials in fp32 (the gather step for row-sharded matmul).

The tiny spectral-norm power-iteration scales (identical math to the
reference: sigma = ||W^T normalize(W u)||) are computed on host in fp32 and
folded into the weights / a single per-side scale gamma.

Device program (SPMD; per-core weight slices), per 512-token window w:
  A: qkv^T = W^T x^T, with x^T provided by the host (fp8 for q/k, fp16 for
     v) so no on-device transposes of x are needed. q,k run as fp8
     DoubleRow matmuls (2 d-chunks per pass, 0.5 cyc/row); v in fp16.
     q^T/k^T are stored as fp8 with a zero second slot so the attention
     S matmul can also use DoubleRow; v^T is PE-transposed into V-natural
     tiles whose cols 64:127 are ones (the PV matmul then also emits the
     softmax denominator for free).
  B: per (head, 2-k-block group): S^T = K Q^T via one fp8 DoubleRow matmul
     per k-block into a 2-bank PSUM group; one Exp activation per group
     (no max pass / shift: scores are O(1) so exp() is in fp16 range);
     causal triangle mask multiply on diagonal blocks; O^T accumulated in
     PSUM with the denominator on partitions 64:127; normalize via
     reciprocal+mult.
  C: y_partial = x_att^T-blocks @ W_proj, staged via Pool copies, fp16 DMA.

Engines are in-order, so stage A(w+1) and C(w-1) ops are interleaved into
stage B(w)'s exp-latency bubbles explicitly (ACT is the critical engine:
it runs only the 80 grouped exps).
"""
from contextlib import ExitStack

import numpy as np

import concourse.bass as bass
import concourse.mybir as mybir
from concourse.bass_utils import run_bass_kernel_spmd
from concourse.tile import TileContext

F32 = mybir.dt.float32
F16 = mybir.dt.float16
F8 = mybir.dt.float8e4

N_CORES = 8
BATCH = 2
NTOK = 4096      # flattened b*n
D = 1024
NH = 2           # heads per core
HD = 64
B = 2
NSEQ = 2048
WQ = 512         # token window
NW = NTOK // WQ
NWB = NSEQ // WQ
KB = 128
BETA = 8.0       # host-side fp8 weight pre-scale (keeps W in fp8e4m3 range)

DR = mybir.MatmulPerfMode.DoubleRow
EXP = mybir.ActivationFunctionType.Exp
MUL = mybir.AluOpType.mult


# ---------------------------------------------------------------------------
# Workaround: this walrus build accepts at most ONE sync wait per
# instruction. Hoist extra waits onto single-wait NOPs inserted before.
# ---------------------------------------------------------------------------
def _split_sync_waits(nc, max_waits=1):
    for f in nc.m.functions:
        for blk in f.blocks:
            insts = blk.instructions
            out = []
            changed = False
            for inst in insts:
                si = inst.sync_info
                waits = list(si.on_wait) if si is not None else []
                if len(waits) > max_waits:
                    extra = waits[:-max_waits]
                    for i in range(0, len(extra), max_waits):
                        nop = mybir.InstNoOp(name=f"I-{nc.next_id()}", ins=[],
                                             outs=[], engine=inst.engine)
                        nop.sync_info = mybir.SyncInfo(
                            on_wait=extra[i:i + max_waits], on_update=[])
                        nc.register_instruction(nop, overwrite=True)
                        out.append(nop)
                    si.on_wait = waits[-max_waits:]
                    inst.sync_info = si
                    changed = True
                out.append(inst)
            if changed:
                blk.instructions = out


class _TileContextSplit(TileContext):
    def __exit__(self, exc_type, exc_value, traceback):
        ret = super().__exit__(exc_type, exc_value, traceback)
        if exc_type is None:
            _split_sync_waits(self.nc)
        return ret


def declare_params(nc):
    xb = nc.declare_dram_parameter("xb", [D, NTOK], F16, isOutput=False)
    x8 = nc.declare_dram_parameter("x8", [D, NTOK], F8, isOutput=False)
    w8q = nc.declare_dram_parameter("w8q", [D, NH * HD], F8, isOutput=False)
    w8k = nc.declare_dram_parameter("w8k", [D, NH * HD], F8, isOutput=False)
    wvh = nc.declare_dram_parameter("wvh", [D, NH * HD], F16, isOutput=False)
    wph = nc.declare_dram_parameter("wph", [NH * HD, D], F16, isOutput=False)
    # col 0: gamma (f16); cols 1..128: causal triangle mask
    cst = nc.declare_dram_parameter("cst", [128, 1 + KB], F16, isOutput=False)
    y = nc.declare_dram_parameter("y", [NTOK, D], F16, isOutput=True)
    return xb, x8, w8q, w8k, wvh, wph, cst, y


def _build_body(nc, tc):
    xb, x8, w8q, w8k, wvh, wph, cst, y = declare_params(nc)

    ctx = ExitStack()
    with ctx:
        singles = ctx.enter_context(tc.tile_pool(name="singles", bufs=1))

        # --- constants / weights to SBUF (all on the SP/HWDGE queue; the
        # Pool DGE path costs ~1us of descriptor generation per transfer) ---
        cst_sb = singles.tile([128, 1 + KB], F16)
        tri_sb = cst_sb[:, 1:1 + KB]
        gam_sb = singles.tile([128, 1], F32)

        w8q_sb = singles.tile([128, 8, NH * HD], F8)
        nc.sync.dma_start(out=w8q_sb[:],
                          in_=w8q.rearrange("(c p) m -> p c m", p=128))
        w8k_sb = singles.tile([128, 8, NH * HD], F8)
        nc.sync.dma_start(out=w8k_sb[:],
                          in_=w8k.rearrange("(c p) m -> p c m", p=128))
        wvh_sb = singles.tile([128, 8, NH * HD], F16)
        wph_sb = singles.tile([128, D], F16)

        # --- persistent per-window tiles ---
        # q^T/k^T as fp8 with a zero slot (dim1) so the S matmul can run in
        # DoubleRow mode (2 contraction tiles per pass; slot 1 contributes 0)
        qT8 = [singles.tile([128, 2, WQ], F8, name=f"qT8_{w}")
               for w in range(NW)]
        kT8 = [singles.tile([128, 2, WQ], F8, name=f"kT8_{w}")
               for w in range(NW)]
        xaw = [singles.tile([128, WQ], F16, name=f"xa_{w}") for w in range(NW)]
        # V natural layout per (head, batch, k-window): [128 k, 4 kb, v|ones]
        vnat = [[[singles.tile([128, 4, 2 * HD], F16, name=f"vn_{h}_{b}_{g}")
                  for g in range(NWB)] for b in range(B)] for h in range(NH)]

        warm = singles.tile([128, KB], F16)

        # zero the fp8 DoubleRow padding slots and the all-ones denominator
        # columns once, on Pool (window-major so window 0 unblocks first)
        nc.gpsimd.memset(warm[:], 0.0)
        for w in range(NW):
            b, g = divmod(w, NWB)
            nc.gpsimd.memset(qT8[w][:, 1, :], 0.0)
            nc.gpsimd.memset(kT8[w][:, 1, :], 0.0)
            for h in range(NH):
                nc.gpsimd.memset(vnat[h][b][g][:, :, HD:2 * HD], 1.0)

        # --- pools ---
        # PSUM budget (8 banks): s 2x2 + o 2 + aux 2 = 8. The "aux" ring
        # carries all short-lived stage A/C accumulators (q, k, v-natural,
        # proj partials) so they never contend with the S-group ring.
        ps = ctx.enter_context(tc.tile_pool(name="ps", bufs=1, space="PSUM"))
        x8_pool = ctx.enter_context(tc.tile_pool(name="x8p", bufs=2))
        xb_pool = ctx.enter_context(tc.tile_pool(name="xbp", bufs=2))
        a_pool = ctx.enter_context(tc.tile_pool(name="apool", bufs=6))
        den_pool = ctx.enter_context(tc.tile_pool(name="denpool", bufs=3))
        y_pool = ctx.enter_context(tc.tile_pool(name="ypool", bufs=5))

        x8_pre = {}
        xb_pre = {}

        def load_x8(w):
            ws = slice(w * WQ, (w + 1) * WQ)
            x8_t = x8_pool.tile([128, 8, WQ], F8, tag="x8", name="x8_t")
            nc.sync.dma_start(
                out=x8_t[:],
                in_=x8[:, ws].rearrange("(c p) n -> p c n", p=128))
            x8_pre[w] = x8_t

        def load_xb(w):
            ws = slice(w * WQ, (w + 1) * WQ)
            xb_t = xb_pool.tile([128, 8, WQ], F16, tag="xb", name="xb_t")
            nc.sync.dma_start(
                out=xb_t[:],
                in_=xb[:, ws].rearrange("(c p) n -> p c n", p=128))
            xb_pre[w] = xb_t

        def load_x(w):
            load_x8(w)
            load_xb(w)

        load_x8(0)
        nc.sync.dma_start(out=cst_sb[:], in_=cst[:])
        nc.vector.tensor_copy(gam_sb[:], cst_sb[:, 0:1])
        nc.sync.dma_start(out=wvh_sb[:],
                          in_=wvh.rearrange("(c p) m -> p c m", p=128))
        load_xb(0)
        nc.sync.dma_start(out=wph_sb[:], in_=wph[:])

        # ---- Stage A, as lists of (pe_ns_estimate, closure) so the emitter
        # can pace them into stage B's exp-latency bubbles. q/k (fp8
        # DoubleRow) only need x8; v (natural layout, f16) only needs xb. ----
        def ops_A_qk(w):
            st = {}
            ops = []

            def op_load():
                if w not in x8_pre:
                    load_x(w)
            ops.append((0, op_load))

            def mk_qk(w8_sb, dst):
                def op_mm():
                    p = ps.tile([128, WQ], F32, tag="aux", name="qkp",
                                bufs=2)
                    for cp in range(4):
                        nc.tensor.matmul(
                            p[:], w8_sb[:, 2 * cp:2 * cp + 2, :],
                            x8_pre[w][:, 2 * cp:2 * cp + 2, :],
                            start=(cp == 0), stop=(cp == 3), perf_mode=DR)
                    st['p'] = p

                def op_cp():
                    nc.vector.tensor_scalar_mul(dst[:, 0, :], st['p'][:],
                                                gam_sb)
                return [(430, op_mm), (60, op_cp)]

            ops += mk_qk(w8q_sb, qT8[w])
            ops += mk_qk(w8k_sb, kT8[w])
            return ops

        def ops_A_v(w):
            # v in natural [token, head-col] layout via xb as the stationary
            # operand: no transposes needed for the PV lhsT
            b, g = divmod(w, NWB)
            ops = []

            def mk_v(half):
                def op():
                    vn_ps = ps.tile([128, 2, NH * HD], F32, tag="aux",
                                    name="vn_ps", bufs=2)
                    for tb in range(2 * half, 2 * half + 2):
                        for c in range(8):
                            nc.tensor.matmul(
                                vn_ps[:, tb - 2 * half, :],
                                xb_pre[w][:, c, tb * KB:(tb + 1) * KB],
                                wvh_sb[:, c, :],
                                start=(c == 0), stop=(c == 7))
                    for h in range(NH):
                        nc.vector.tensor_copy(
                            vnat[h][b][g][:, 2 * half:2 * half + 2, 0:HD],
                            vn_ps[:, :, h * HD:(h + 1) * HD])
                    if half == 1:
                        x8_pre.pop(w)
                        xb_pre.pop(w)
                return op
            ops.append((900, mk_v(0)))
            ops.append((900, mk_v(1)))
            return ops

        # ---- Stage C for one token window: proj partials, 4 n-blocks.
        # The last window's copies alternate DVE/ACT and DMA per half so the
        # post-last-exp tail drains two engines wide. ----
        # Tail variant of stage C: the final window's staging drains after
        # the last exp with every engine idle, so batch each n-block through
        # a free 2-bank "s" tile and alternate the big copies DVE/ACT.
        def ops_C_tail(w):
            ops = []
            for nb_loc in range(4):
                def op_mm(nb_loc=nb_loc):
                    yp = ps.tile([128, 2, WQ], F32, tag="s", name="ypt",
                                 bufs=2)
                    for cc in range(2):
                        nc.tensor.matmul(
                            yp[:, cc, :],
                            xaw[w][:, nb_loc * 128:(nb_loc + 1) * 128],
                            wph_sb[:, cc * WQ:(cc + 1) * WQ],
                            start=True, stop=True)
                    y_sb = y_pool.tile([128, D], F16, tag="y", name="y_sbt")
                    nc.vector.tensor_copy(y_sb[:, 0:WQ], yp[:, 0, :])
                    nc.scalar.copy(y_sb[:, WQ:D], yp[:, 1, :])
                    nb = 4 * w + nb_loc
                    nc.sync.dma_start(out=y[nb * 128:(nb + 1) * 128, :],
                                      in_=y_sb[:])
                ops.append((450, op_mm))
            return ops

        def ops_C(w):
            st = {}
            ops = []
            for nb_loc in range(4):
                for cc in range(2):
                    def op_mm(nb_loc=nb_loc, cc=cc):
                        yp = ps.tile([128, WQ], F32, tag="aux", name="yp",
                                     bufs=2)
                        nc.tensor.matmul(
                            yp[:],
                            xaw[w][:, nb_loc * 128:(nb_loc + 1) * 128],
                            wph_sb[:, cc * WQ:(cc + 1) * WQ],
                            start=True, stop=True)
                        nb = 4 * w + nb_loc
                        if cc == 0:
                            st['y'] = y_pool.tile([128, D], F16, tag="y",
                                                  name="y_sb")
                        y_sb = st['y']
                        nc.vector.tensor_copy(
                            y_sb[:, cc * WQ:(cc + 1) * WQ], yp[:])
                        if cc == 1:
                            nc.sync.dma_start(
                                out=y[nb * 128:(nb + 1) * 128, :],
                                in_=y_sb[:])
                    ops.append((250, op_mm))
            return ops

        # ---- Stage B: globally software-pipelined over unit pairs. Each
        # pair's PVs (and, for the last pair of a (b,g), the reciprocal
        # normalization) are deferred into the NEXT pair's emission so the
        # in-order PE queue never parks on a just-issued exp — including
        # across (b,g) boundaries. ----
        pend_pv = [None]

        # Global pair-stream filler schedule: pair p (0..39) across all
        # (b,g) gets sched[p] closures; stage-C closures drain from a global
        # queue at a rate that spreads them over the remaining pairs.
        n_pairs_w = [2 * (divmod(w, NWB)[1] + 1) for w in range(NW)]
        pair_start = [sum(n_pairs_w[:w]) for w in range(NW + 1)]
        total_pairs = pair_start[NW]
        sched = [[] for _ in range(total_pairs)]
        carry_c = []
        pair_idx = [0]

        def place(items, lo, hi):
            # spread items evenly over pairs [lo, hi), preserving order
            n = len(items)
            span = max(1, hi - lo)
            for i, it in enumerate(items):
                p = min(hi - 1, lo + i * span // n)
                sched[max(0, p)].append(it)

        def stage_B(b, g):
            w = b * NWB + g
            n_pairs = 2 * (g + 1)
            st = {}

            def emit_pv(u, a_ts):
                kw, j = divmod(u, 2)
                diag = (kw == g)
                if u == 0:
                    st['o'] = [ps.tile([128, WQ], F32, tag="o",
                                       name=f"o_ps{h}", bufs=2)
                               for h in range(NH)]
                o_ps = st['o']
                for h in range(NH):
                    a_t = a_ts[h]
                    if diag:
                        for t in range(2):
                            c0 = (2 * j + t) * KB
                            nc.gpsimd.tensor_tensor(
                                out=a_t[:, t, c0:c0 + KB],
                                in0=a_t[:, t, c0:c0 + KB],
                                in1=tri_sb, op=MUL)
                    for t in range(2):
                        kloc = 2 * j + t
                        sq = kloc * KB if diag else 0
                        nc.tensor.matmul(
                            o_ps[h][:, sq:WQ] if sq else o_ps[h][:],
                            vnat[h][b][kw][:, kloc, :],
                            a_t[:, t, sq:WQ] if sq else a_t[:, t, :],
                            start=(u == 0 and t == 0),
                            stop=(u == n_pairs - 1 and t == 1))
                if u == n_pairs - 1:
                    if w == NW - 1:
                        # final window: normalize per token-half so the tail
                        # proj can start before the second half is done
                        for tk in range(2):
                            ts = slice(tk * 256, (tk + 1) * 256)
                            for h in range(NH):
                                hs = slice(h * HD, (h + 1) * HD)
                                den = den_pool.tile([HD, 256], F32,
                                                    tag="denh", name="denh")
                                nc.vector.reciprocal(
                                    den[:], o_ps[h][HD:2 * HD, ts])
                                nc.vector.tensor_tensor(
                                    out=xaw[w][hs, ts],
                                    in0=o_ps[h][0:HD, ts],
                                    in1=den[:], op=MUL)
                    else:
                        for h in range(NH):
                            hs = slice(h * HD, (h + 1) * HD)
                            den = den_pool.tile([HD, WQ], F32, tag="den",
                                                name="den")
                            nc.vector.reciprocal(den[:],
                                                 o_ps[h][HD:2 * HD, :])
                            nc.vector.tensor_tensor(out=xaw[w][hs, :],
                                                    in0=o_ps[h][0:HD, :],
                                                    in1=den[:], op=MUL)

            for u in range(n_pairs):
                kw, j = divmod(u, 2)
                kwin = b * NWB + kw
                # columns below 2j*KB of a diagonal group are causally dead:
                # skip them in both the S matmuls and the exp
                c0 = 2 * j * KB if kw == g else 0
                a_ts = [None, None]
                for h in range(NH):
                    hs = slice(h * HD, (h + 1) * HD)
                    s_grp = ps.tile([128, 2, WQ], F32, tag="s", name="s_grp",
                                    bufs=2)
                    for t in range(2):
                        kloc = 2 * j + t
                        nc.tensor.matmul(
                            s_grp[:, t, c0:WQ],
                            kT8[kwin][hs, :, kloc * KB:(kloc + 1) * KB],
                            qT8[w][hs, :, c0:WQ],
                            start=True, stop=True, perf_mode=DR)
                    a_t = a_pool.tile([128, 2, WQ], F16, tag="a", name="a_t")
                    nc.scalar.activation(a_t[:, :, c0:WQ],
                                         s_grp[:, :, c0:WQ], EXP)
                    a_ts[h] = a_t
                # previous pair's PVs (and possibly the previous window's
                # finale) FIRST: stage-C closures behind it read xaw, and the
                # tile dep tracker only orders reads after already-emitted
                # writes
                if pend_pv[0] is not None:
                    pend_pv[0]()
                pend_pv[0] = (lambda u=u, a_ts=a_ts: emit_pv(u, a_ts))
                # filler into the exp latency bubble: this pair's scheduled
                # stage-A closures, then stage-C at the global drain rate
                p = pair_idx[0]
                for _, fn in sched[p]:
                    fn()
                want_c = -(-len(carry_c) // max(1, total_pairs - p))
                for _ in range(min(want_c, 2, len(carry_c))):
                    carry_c.pop(0)[1]()
                pair_idx[0] = p + 1

        def riffle(a, c):
            out = []
            n = max(len(a), len(c))
            for i in range(n):
                if i < len(a):
                    out.append(a[i])
                if i < len(c):
                    out.append(c[i])
            return out

        # ---- software-pipelined emission ----
        # A_qk(0) up front (x8-only, lands early); v(0) in the first pairs
        # so the PE queue never parks on the larger xb DMA. A(w) is placed
        # in the ~7 pairs preceding B(w)'s start (crossing (b,g) boundaries
        # so short windows don't choke on filler).
        for _, op in ops_A_qk(0):
            op()
        place(ops_A_v(0), 0, 2)
        for w in range(1, NW):
            place(ops_A_qk(w) + ops_A_v(w),
                  max(0, pair_start[w] - 7), pair_start[w])
        for w in range(NW):
            b, g = divmod(w, NWB)
            if w >= 1:
                carry_c.extend(ops_C(w - 1))
            stage_B(b, g)
        pend_pv[0]()
        for _, op in carry_c:
            op()
        for _, op in ops_C_tail(NW - 1):
            op()


def _host_scales(W_qkv, u_qkv, sigma_qkv, W_proj, u_proj, sigma_proj):
    """Power-iteration spectral norm in fp32, exactly as the reference:
    v = normalize(W u); sigma = ||W^T v||."""
    def sig(W, u):
        v = (W @ u).astype(np.float32)
        v = v / np.float32(np.linalg.norm(v))
        u2 = (W.T @ v).astype(np.float32)
        return np.float32(np.linalg.norm(u2))
    c_qkv = np.float32(sigma_qkv[0]) / sig(W_qkv, u_qkv)
    c_proj = np.float32(sigma_proj[0]) / sig(W_proj, u_proj)
    return np.float32(c_qkv), np.float32(c_proj)


def _make_tri():
    p = np.arange(128)[:, None]
    q = np.arange(KB)[None, :]
    return (q >= p).astype(np.float32)


def make_in_maps(batch, W_qkv, u_qkv, sigma_qkv, W_proj, u_proj, sigma_proj):
    import ml_dtypes
    f16 = np.float16
    f8 = ml_dtypes.float8_e4m3
    batch = np.asarray(batch, np.float32)
    W_qkv = np.asarray(W_qkv, np.float32)
    u_qkv = np.asarray(u_qkv, np.float32)
    sigma_qkv = np.asarray(sigma_qkv, np.float32)
    W_proj = np.asarray(W_proj, np.float32)
    u_proj = np.asarray(u_proj, np.float32)
    sigma_proj = np.asarray(sigma_proj, np.float32)
    c_qkv, c_proj = _host_scales(W_qkv, u_qkv, sigma_qkv,
                                 W_proj, u_proj, sigma_proj)
    xT = np.ascontiguousarray(batch.reshape(NTOK, D).T)
    xb_h = xT.astype(f16)
    x8_h = xT.astype(f8)
    # S needs scale c^2/sqrt(hd); gamma is the per-side share applied at the
    # q^T/k^T PSUM->SBUF copies (fp8 weights carry a BETA pre-scale to stay
    # in e4m3's normal range)
    # (gamma*BETA)^2 == c^2 / sqrt(hd)  =>  gamma = c / (BETA * hd^(1/4))
    gamma = np.float32(c_qkv / (BETA * HD ** 0.25))
    cst = np.concatenate(
        [np.full((128, 1), gamma, np.float32), _make_tri()],
        axis=1).astype(f16)
    cst = np.ascontiguousarray(cst)
    in_maps = []
    for c in range(N_CORES):
        cs = slice(128 * c, 128 * (c + 1))
        w8q_h = np.ascontiguousarray((W_qkv[:, cs] * BETA).astype(f8))
        w8k_h = np.ascontiguousarray(
            (W_qkv[:, 1024 + 128 * c:1024 + 128 * (c + 1)] * BETA).astype(f8))
        wvh_h = np.ascontiguousarray(
            (W_qkv[:, 2048 + 128 * c:2048 + 128 * (c + 1)] * c_qkv)
            .astype(f16))
        wph_h = np.ascontiguousarray((W_proj[cs, :] * c_proj).astype(f16))
        in_maps.append({
            "xb": xb_h,
            "x8": x8_h,
            "w8q": w8q_h,
            "w8k": w8k_h,
            "wvh": wvh_h,
            "wph": wph_h,
            "cst": cst,
        })
    return in_maps


_NC_CACHE = None


def build_nc():
    global _NC_CACHE
    if _NC_CACHE is None:
        nc = bass.Bass("TRN2", target_bir_lowering=False, debug=False,
                       num_devices=N_CORES)
        with _TileContextSplit(nc) as tc:
            _build_body(nc, tc)
        _NC_CACHE = nc
    return _NC_CACHE


def kernel(batch, W_qkv, u_qkv, sigma_qkv, W_proj, u_proj, sigma_proj):
    in_maps = make_in_maps(batch, W_qkv, u_qkv, sigma_qkv,
                           W_proj, u_proj, sigma_proj)
    nc = build_nc()
    res = run_bass_kernel_spmd(nc, in_maps, list(range(N_CORES)))
    y = np.zeros((NTOK, D), np.float32)
    for c in range(N_CORES):
        y += res.results[c]["y"].astype(np.float32)
    return y.reshape(BATCH, NSEQ, D)
